# revision 14
# baseline (speedup 1.0000x reference)
"""Nearest-neighbor retrieval kernel for Trainium2 (8 NeuronCores, SPMD).

Problem: dis[i] = mean((in_vel - train_obs_vel[i])**2); return
train_target_vel[argmin(dis)].

Strategy: only train_obs_vel has to stream through the devices. The device
pass is a bf16 *screen*: it computes approximate keys
k_i ~= sum((x_i - q)^2) for every sample (bf16 halves HBM traffic to
~26.5 MB/core and doubles VectorE throughput). The host then recomputes
exact f32 keys for the top-1024 screened candidates (~1 M flops) and picks
the true argmin — bf16 key noise is ~+-0.3 on a min-gap of ~4, so the true
argmin is inside the top-1024 with overwhelming margin, and the final
result is bit-exact.

Sharding: 12500 rows per core, padded to 12544 = 128*98 so DMA tiles have
128 partitions (the HWDGE only engages all 16 SDMA engines for
128-partition descriptor lists; at 125 partitions it uses 5 and runs 3x
slower). Per column (sample-slice) the engines split work:
  - VectorE  TENSOR_TENSOR(subtract) bf16 (2x mode)   -> diff
  - ScalarE  ACTIVATE(Square, accum_out)              -> key (most cols)
  - VectorE  TENSOR_TENSOR(mult) + TENSOR_REDUCE(add) -> key (offload cols)
so ScalarE (dtype-independent 1 elem/cycle) stops being the bottleneck.
The tiny [128, 98] f32 key tile per core returns to the host; no device
collectives are needed.
"""

import sys

sys.path.insert(0, "/opt/trn_rl_repo")

import ml_dtypes
import numpy as np

import concourse.bacc as bacc
import concourse.mybir as mybir
import concourse.tile as tile
from concourse.bass_utils import run_bass_kernel_spmd

# Problem shapes (hardcoded per harness contract)
N = 100000
T_OBS = 16
T_OUT = 25
D = 66
F = T_OBS * D  # 1056 features per sample
CORES = 8
PER = N // CORES  # 12500 samples per core
P = 128  # SBUF partitions
C = 98  # samples (columns) per partition
PER_PAD = P * C  # 12544 padded samples per core
PAD_VAL = 1.0e4  # pad rows get a huge distance; never the argmin
S = 2  # samples per partition per DMA tile
N_VCOLS = 27  # columns whose square+reduce runs on VectorE (mul+reduce)
N_GSUBS = 36  # columns whose subtract runs on GpSimd instead of VectorE
TOPK = 1024  # host-side exact recheck pool


def _spread(n, total=C):
    """n column indices spread evenly over [0, total)."""
    return {int(round(i * total / n)) % total for i in range(n)}

_f32 = mybir.dt.float32
_bf16 = mybir.dt.bfloat16
_bf16_np = ml_dtypes.bfloat16


def build_nc(s=S, xin_bufs=8, n_vcols=N_VCOLS, n_gsubs=N_GSUBS):
    ntiles = C // s
    assert ntiles * s == C
    vcols = _spread(n_vcols)
    gsubs = _spread(n_gsubs)
    nc = bacc.Bacc("TRN2", target_bir_lowering=False, debug=False)
    x = nc.dram_tensor("x", [PER_PAD, F], _bf16, kind="ExternalInput")
    qb = nc.dram_tensor("qb", [P, F], _bf16, kind="ExternalInput")
    key_out = nc.dram_tensor("key", [P, C], _f32, kind="ExternalOutput")

    # [12544, 1056] -> [128 partitions, 98*1056 contiguous bf16]
    xr = x[:].rearrange("(p c) d -> p (c d)", p=P)

    with tile.TileContext(nc) as tc:
        with (
            tc.tile_pool(name="xin", bufs=xin_bufs) as xpool,
            tc.tile_pool(name="qpool", bufs=1) as qpool,
            tc.tile_pool(name="scratch", bufs=4) as spool,
            tc.tile_pool(name="acc", bufs=1) as apool,
        ):
            q_tile = qpool.tile([P, F], _bf16)
            nc.sync.dma_start(out=q_tile[:], in_=qb[:])

            key_t = apool.tile([P, C], _f32)

            for t in range(ntiles):
                xt = xpool.tile([P, s * F], _bf16, tag="xt")
                nc.sync.dma_start(
                    out=xt[:], in_=xr[:, t * s * F : (t + 1) * s * F]
                )
                for j in range(s):
                    col = t * s + j
                    xs = xt[:, j * F : (j + 1) * F]
                    diff = spool.tile([P, F], _bf16, tag="diff")
                    sub_eng = nc.gpsimd if col in gsubs else nc.vector
                    sub_eng.tensor_sub(diff[:], xs, q_tile[:])
                    kcol = key_t[:, col : col + 1]
                    if col in vcols:
                        # VectorE path: mult + reduce
                        sq = spool.tile([P, F], _bf16, tag="vsq")
                        nc.vector.tensor_mul(sq[:], diff[:], diff[:])
                        nc.vector.tensor_reduce(
                            kcol,
                            sq[:],
                            axis=mybir.AxisListType.X,
                            op=mybir.AluOpType.add,
                        )
                    else:
                        # ScalarE path: Square with free-axis accumulate
                        sq = spool.tile([P, F], _bf16, tag="ssq")
                        nc.scalar.activation(
                            out=sq[:],
                            in_=diff[:],
                            func=mybir.ActivationFunctionType.Square,
                            accum_out=kcol,
                        )

            nc.sync.dma_start(out=key_out[:], in_=key_t[:])
    nc.compile()
    return nc


_nc_cache = {}


def _get_nc():
    key = (S, N_VCOLS, N_GSUBS)
    if key not in _nc_cache:
        _nc_cache[key] = build_nc()
    return _nc_cache[key]


def make_in_maps(in_vel, train_obs_vel):
    q = np.asarray(in_vel, dtype=np.float32).reshape(F)
    qbn = np.ascontiguousarray(
        np.broadcast_to(q.astype(_bf16_np), (P, F))
    )
    X = np.asarray(train_obs_vel, dtype=np.float32).reshape(N, F)
    Xb = X.astype(_bf16_np)
    in_maps = []
    for c in range(CORES):
        xp = np.full((PER_PAD, F), PAD_VAL, dtype=_bf16_np)
        xp[:PER] = Xb[c * PER : (c + 1) * PER]
        in_maps.append({"x": xp, "qb": qbn})
    return in_maps


def finish(results, in_vel, train_obs_vel, train_target_vel):
    # keys[core][p, col] screens padded-local sample p*C + col; flattening
    # in C order reproduces the padded-local sample order.
    keys = np.stack([np.asarray(r["key"]) for r in results])  # [8, P, C]
    flat = keys.reshape(CORES, PER_PAD)[:, :PER].reshape(-1)  # drop pads
    k = min(TOPK, flat.size)
    cand = np.sort(np.argpartition(flat, k - 1)[:k])
    # exact f32 recheck of the screened candidates
    q = np.asarray(in_vel, dtype=np.float32).reshape(F)
    X = np.asarray(train_obs_vel, dtype=np.float32).reshape(N, F)
    d = X[cand] - q
    exact = np.einsum("ij,ij->i", d, d)
    best = int(cand[int(exact.argmin())])
    out = np.asarray(train_target_vel)[best]
    return np.ascontiguousarray(out)


def kernel(in_vel, train_obs_vel, train_target_vel):
    nc = _get_nc()
    in_maps = make_in_maps(in_vel, train_obs_vel)
    res = run_bass_kernel_spmd(nc, in_maps, list(range(CORES)))
    return finish(res.results, in_vel, train_obs_vel, train_target_vel)


# revision 17
# speedup vs baseline: 1.1846x; 1.1846x over previous
"""Nearest-neighbor retrieval kernel for Trainium2 (8 NeuronCores, SPMD).

Problem: dis[i] = mean((in_vel - train_obs_vel[i])**2); return
train_target_vel[argmin(dis)].

Strategy: only train_obs_vel has to stream through the devices. The device
pass is a bf16 *screen* over a host-transposed layout XT[feature, sample]:
with features on partitions, the query q becomes a per-partition scalar, so

  - ScalarE  ACTIVATE(Square, bias=-q)           -> (x-q)^2 in ONE op
  - VectorE  TENSOR_SCALAR(sub) + TENSOR_TENSOR(mult)  (2x bf16 mode)
  - TensorE  ones-vector matmul, PSUM-accumulated over the 9 feature
             chunks                              -> sum over features

The feature reduction runs on the otherwise-idle TensorE instead of
costing VectorE/ScalarE passes, and the subtract folds into the ACT op.
The screen keys (~0.4% noise vs a ~0.2% min-gap margin) feed a host-side
exact f32 recheck of the top-1024 candidates, so the result is bit-exact.

Sharding: 12500 samples per core, padded to 12544; features padded
1056 -> chunks of 128x8 + 32. All big DMAs use 128-partition descriptor
lists (the HWDGE only engages all 16 SDMA engines at 128 partitions).
"""

import sys

sys.path.insert(0, "/opt/trn_rl_repo")

import ml_dtypes
import numpy as np

import concourse.bacc as bacc
import concourse.mybir as mybir
import concourse.tile as tile
from concourse.bass_utils import run_bass_kernel_spmd

# Problem shapes (hardcoded per harness contract)
N = 100000
T_OBS = 16
T_OUT = 25
D = 66
F = T_OBS * D  # 1056 features per sample
CORES = 8
PER = N // CORES  # 12500 samples per core
NS = 12544  # padded samples per core (128*98)
PAD_VAL = 1.0e4  # pad samples get a huge distance; never the argmin
NCH = 9  # feature chunks: 8 x 128 + 1 x 32
PCH = [128] * 8 + [32]
W = 448  # samples per PSUM block (one 2KB f32 bank)
NBLK = NS // W  # 28 blocks
BPG = 7  # blocks per DMA sample-group
DW = W * BPG  # 3136 samples per DMA group
NGRP = NBLK // BPG  # 4 groups
S_OF_9 = 5  # chunks per block whose square runs on ScalarE (rest VectorE)
TOPK = 1024  # host-side exact recheck pool

_f32 = mybir.dt.float32
_bf16 = mybir.dt.bfloat16
_bf16_np = ml_dtypes.bfloat16


def build_nc(s_of_9=S_OF_9):
    nc = bacc.Bacc("TRN2", target_bir_lowering=False, debug=False)
    xt = nc.dram_tensor("xt", [F, NS], _bf16, kind="ExternalInput")
    qn = nc.dram_tensor("qn", [128, NCH], _f32, kind="ExternalInput")  # -q
    qp = nc.dram_tensor("qp", [128, NCH], _f32, kind="ExternalInput")  # +q
    key_out = nc.dram_tensor("key", [1, NS], _f32, kind="ExternalOutput")

    with tile.TileContext(nc) as tc:
        with (
            tc.tile_pool(name="xin", bufs=2) as xpool,
            tc.tile_pool(name="qpool", bufs=1) as qpool,
            tc.tile_pool(name="scratch", bufs=6) as spool,
            tc.tile_pool(name="kout", bufs=4) as kpool,
            tc.tile_pool(name="psum", bufs=4, space="PSUM") as ppool,
        ):
            qn_t = qpool.tile([128, NCH], _f32, tag="qn")
            nc.sync.dma_start(out=qn_t[:], in_=qn[:])
            qp_t = qpool.tile([128, NCH], _f32, tag="qp")
            nc.sync.dma_start(out=qp_t[:], in_=qp[:])
            ones = qpool.tile([128, 1], _bf16, tag="ones")
            nc.vector.memset(ones[:], 1.0)

            dma_engs = [nc.sync, nc.gpsimd]
            dma_i = 0
            blk = 0
            for g in range(NGRP):
                # load all 9 feature-chunk strips for this sample group
                xts = []
                for c in range(NCH):
                    p = PCH[c]
                    xtile = xpool.tile([128, DW], _bf16, tag=f"x{c}")
                    eng = dma_engs[dma_i % len(dma_engs)]
                    dma_i += 1
                    eng.dma_start(
                        out=xtile[0:p, :],
                        in_=xt[c * 128 : c * 128 + p, g * DW : (g + 1) * DW],
                    )
                    xts.append(xtile)

                for b in range(BPG):
                    lo, hi = b * W, (b + 1) * W
                    psum = ppool.tile([1, W], _f32)
                    for c in range(NCH):
                        p = PCH[c]
                        xs = xts[c][0:p, lo:hi]
                        sq = spool.tile([128, W], _bf16, tag="sq")
                        if (c + blk) % NCH < s_of_9:
                            # ScalarE: (x - q)^2 in one ACTIVATE
                            nc.scalar.activation(
                                out=sq[0:p, :],
                                in_=xs,
                                func=mybir.ActivationFunctionType.Square,
                                bias=qn_t[0:p, c : c + 1],
                            )
                        else:
                            # VectorE: subtract (2x) + square (2x)
                            d = spool.tile([128, W], _bf16, tag="d")
                            nc.vector.tensor_scalar_sub(
                                d[0:p, :], xs, qp_t[0:p, c : c + 1]
                            )
                            nc.vector.tensor_mul(sq[0:p, :], d[0:p, :], d[0:p, :])
                        nc.tensor.matmul(
                            psum[:],
                            ones[0:p, :],
                            sq[0:p, :],
                            start=(c == 0),
                            stop=(c == NCH - 1),
                        )
                    # PSUM -> SBUF -> DRAM
                    kt = kpool.tile([1, W], _f32, tag="kt")
                    if blk % 2 == 0:
                        nc.scalar.copy(kt[:], psum[:])
                    else:
                        nc.vector.tensor_copy(kt[:], psum[:])
                    nc.gpsimd.dma_start(
                        out=key_out[:, g * DW + lo : g * DW + hi], in_=kt[:]
                    )
                    blk += 1
    nc.compile()
    return nc


_nc_cache = {}


def _get_nc():
    key = (S_OF_9,)
    if key not in _nc_cache:
        _nc_cache[key] = build_nc()
    return _nc_cache[key]


def make_in_maps(in_vel, train_obs_vel):
    q = np.asarray(in_vel, dtype=np.float32).reshape(F)
    qpad = np.zeros(128 * NCH, dtype=np.float32)
    qpad[:F] = q
    qcols = qpad.reshape(NCH, 128).T  # [128, NCH]
    qp_h = np.ascontiguousarray(qcols)
    qn_h = np.ascontiguousarray(-qcols)
    X = np.asarray(train_obs_vel, dtype=np.float32).reshape(N, F)
    in_maps = []
    for c in range(CORES):
        xb = np.full((F, NS), PAD_VAL, dtype=_bf16_np)
        xb[:, :PER] = X[c * PER : (c + 1) * PER].astype(_bf16_np).T
        in_maps.append({"xt": np.ascontiguousarray(xb), "qn": qn_h, "qp": qp_h})
    return in_maps


def finish(results, in_vel, train_obs_vel, train_target_vel):
    # keys[core][0, j] screens padded-local sample j (original column order)
    keys = np.stack([np.asarray(r["key"]) for r in results])  # [8, 1, NS]
    flat = keys.reshape(CORES, NS)[:, :PER].reshape(-1)  # drop pads
    k = min(TOPK, flat.size)
    cand = np.sort(np.argpartition(flat, k - 1)[:k])
    # exact f32 recheck of the screened candidates
    q = np.asarray(in_vel, dtype=np.float32).reshape(F)
    X = np.asarray(train_obs_vel, dtype=np.float32).reshape(N, F)
    d = X[cand] - q
    exact = np.einsum("ij,ij->i", d, d)
    best = int(cand[int(exact.argmin())])
    out = np.asarray(train_target_vel)[best]
    return np.ascontiguousarray(out)


def kernel(in_vel, train_obs_vel, train_target_vel):
    nc = _get_nc()
    in_maps = make_in_maps(in_vel, train_obs_vel)
    res = run_bass_kernel_spmd(nc, in_maps, list(range(CORES)))
    return finish(res.results, in_vel, train_obs_vel, train_target_vel)


# revision 19
# speedup vs baseline: 1.2267x; 1.0355x over previous
"""Nearest-neighbor retrieval kernel for Trainium2 (8 NeuronCores, SPMD).

Problem: dis[i] = mean((in_vel - train_obs_vel[i])**2); return
train_target_vel[argmin(dis)].

Strategy: only train_obs_vel has to stream through the devices. The device
pass is a bf16 *screen* over a host-transposed layout XT[feature, sample]:
with features on partitions, the query q becomes a per-partition scalar, so

  - ScalarE  ACTIVATE(Square, bias=-q)           -> (x-q)^2 in ONE op
  - VectorE  TENSOR_SCALAR(sub) + TENSOR_TENSOR(mult)  (2x bf16 mode)
  - TensorE  ones-vector matmul, PSUM-accumulated over the 9 feature
             chunks                              -> sum over features

The feature reduction runs on the otherwise-idle TensorE instead of
costing VectorE/ScalarE passes, and the subtract folds into the ACT op.
The screen keys (~0.4% noise vs a ~0.2% min-gap margin) feed a host-side
exact f32 recheck of the top-1024 candidates, so the result is bit-exact.

Sharding: 12500 samples per core, padded to 12544; features padded
1056 -> chunks of 128x8 + 32. All big DMAs use 128-partition descriptor
lists (the HWDGE only engages all 16 SDMA engines at 128 partitions).
"""

import sys

sys.path.insert(0, "/opt/trn_rl_repo")

import ml_dtypes
import numpy as np

import concourse.bacc as bacc
import concourse.mybir as mybir
import concourse.tile as tile
from concourse.bass_utils import run_bass_kernel_spmd

# Problem shapes (hardcoded per harness contract)
N = 100000
T_OBS = 16
T_OUT = 25
D = 66
F = T_OBS * D  # 1056 features per sample
CORES = 8
PER = N // CORES  # 12500 samples per core
NS = 12544  # padded samples per core (128*98)
PAD_VAL = 1.0e4  # pad samples get a huge distance; never the argmin
NCH = 9  # feature chunks: 8 x 128 + 1 x 32
PCH = [128] * 8 + [32]
W = 448  # samples per PSUM block (one 2KB f32 bank)
NBLK = NS // W  # 28 blocks
BPG = 7  # blocks per DMA sample-group
DW = W * BPG  # 3136 samples per DMA group
NGRP = NBLK // BPG  # 4 groups
S_OF_9 = 5  # chunks per block whose square runs on ScalarE (rest VectorE)
TOPK = 1024  # host-side exact recheck pool

_f32 = mybir.dt.float32
_bf16 = mybir.dt.bfloat16
_bf16_np = ml_dtypes.bfloat16


def build_nc(s_of_9=S_OF_9):
    nc = bacc.Bacc("TRN2", target_bir_lowering=False, debug=False)
    xt = nc.dram_tensor("xt", [F, NS], _bf16, kind="ExternalInput")
    qn = nc.dram_tensor("qn", [128, NCH], _f32, kind="ExternalInput")  # -q
    qp = nc.dram_tensor("qp", [128, NCH], _f32, kind="ExternalInput")  # +q
    key_out = nc.dram_tensor("key", [1, NS], _f32, kind="ExternalOutput")

    with tile.TileContext(nc) as tc:
        with (
            tc.tile_pool(name="xin", bufs=2) as xpool,
            tc.tile_pool(name="qpool", bufs=1) as qpool,
            tc.tile_pool(name="scratch", bufs=6) as spool,
            tc.tile_pool(name="kout", bufs=4) as kpool,
            tc.tile_pool(name="psum", bufs=4, space="PSUM") as ppool,
        ):
            qn_t = qpool.tile([128, NCH], _f32, tag="qn")
            nc.sync.dma_start(out=qn_t[:], in_=qn[:])
            qp_t = qpool.tile([128, NCH], _f32, tag="qp")
            nc.sync.dma_start(out=qp_t[:], in_=qp[:])
            ones = qpool.tile([128, 1], _bf16, tag="ones")
            nc.vector.memset(ones[:], 1.0)

            pair_i = 0
            for g in range(NGRP):
                # load all 9 feature-chunk strips for this sample group
                xts = []
                for c in range(NCH):
                    p = PCH[c]
                    xtile = xpool.tile([128, DW], _bf16, tag=f"x{c}")
                    nc.sync.dma_start(
                        out=xtile[0:p, :],
                        in_=xt[c * 128 : c * 128 + p, g * DW : (g + 1) * DW],
                    )
                    xts.append(xtile)

                # sq production in 2-block pairs (bigger ops); matmuls per block
                for b0, b1 in ((0, 2), (2, 4), (4, 6), (6, 7)):
                    lo, hi = b0 * W, b1 * W
                    ww = hi - lo
                    # S-chunks alternate 5/6 of 9 to balance ScalarE/VectorE
                    n_s = s_of_9 + (pair_i % 2)
                    pair_i += 1
                    sqs = []
                    for c in range(NCH):
                        p = PCH[c]
                        xs = xts[c][0:p, lo:hi]
                        sq = spool.tile([128, 2 * W], _bf16, tag="sq")
                        if c < n_s:
                            # ScalarE: (x - q)^2 in one ACTIVATE
                            nc.scalar.activation(
                                out=sq[0:p, 0:ww],
                                in_=xs,
                                func=mybir.ActivationFunctionType.Square,
                                bias=qn_t[0:p, c : c + 1],
                            )
                        else:
                            # VectorE: subtract (2x) + square (2x)
                            d = spool.tile([128, 2 * W], _bf16, tag="d")
                            nc.vector.tensor_scalar_sub(
                                d[0:p, 0:ww], xs, qp_t[0:p, c : c + 1]
                            )
                            nc.vector.tensor_mul(
                                sq[0:p, 0:ww], d[0:p, 0:ww], d[0:p, 0:ww]
                            )
                        sqs.append(sq)
                    for b in range(b0, b1):
                        slo = (b - b0) * W
                        # two parallel PSUM chains: S-chunks -> A, V-chunks -> B
                        psa = ppool.tile([1, W], _f32, tag="psa")
                        psb = ppool.tile([1, W], _f32, tag="psb")
                        for c in range(NCH):
                            p = PCH[c]
                            ps, group = (
                                (psa, (0, n_s - 1))
                                if c < n_s
                                else (psb, (n_s, NCH - 1))
                            )
                            nc.tensor.matmul(
                                ps[:],
                                ones[0:p, :],
                                sqs[c][0:p, slo : slo + W],
                                start=(c == group[0]),
                                stop=(c == group[1]),
                            )
                        # merge chains: ScalarE copies B out of PSUM (TT can
                        # only read one PSUM input), VectorE adds
                        ktb = kpool.tile([1, W], _f32, tag="ktb")
                        nc.scalar.copy(ktb[:], psb[:])
                        kt = kpool.tile([1, W], _f32, tag="kt")
                        nc.vector.tensor_add(kt[:], psa[:], ktb[:])
                        nc.gpsimd.dma_start(
                            out=key_out[:, g * DW + b * W : g * DW + (b + 1) * W],
                            in_=kt[:],
                        )
    nc.compile()
    return nc


_nc_cache = {}


def _get_nc():
    key = (S_OF_9,)
    if key not in _nc_cache:
        _nc_cache[key] = build_nc()
    return _nc_cache[key]


def make_in_maps(in_vel, train_obs_vel):
    q = np.asarray(in_vel, dtype=np.float32).reshape(F)
    qpad = np.zeros(128 * NCH, dtype=np.float32)
    qpad[:F] = q
    qcols = qpad.reshape(NCH, 128).T  # [128, NCH]
    qp_h = np.ascontiguousarray(qcols)
    qn_h = np.ascontiguousarray(-qcols)
    X = np.asarray(train_obs_vel, dtype=np.float32).reshape(N, F)
    in_maps = []
    for c in range(CORES):
        xb = np.full((F, NS), PAD_VAL, dtype=_bf16_np)
        xb[:, :PER] = X[c * PER : (c + 1) * PER].astype(_bf16_np).T
        in_maps.append({"xt": np.ascontiguousarray(xb), "qn": qn_h, "qp": qp_h})
    return in_maps


def finish(results, in_vel, train_obs_vel, train_target_vel):
    # keys[core][0, j] screens padded-local sample j (original column order)
    keys = np.stack([np.asarray(r["key"]) for r in results])  # [8, 1, NS]
    flat = keys.reshape(CORES, NS)[:, :PER].reshape(-1)  # drop pads
    k = min(TOPK, flat.size)
    cand = np.sort(np.argpartition(flat, k - 1)[:k])
    # exact f32 recheck of the screened candidates
    q = np.asarray(in_vel, dtype=np.float32).reshape(F)
    X = np.asarray(train_obs_vel, dtype=np.float32).reshape(N, F)
    d = X[cand] - q
    exact = np.einsum("ij,ij->i", d, d)
    best = int(cand[int(exact.argmin())])
    out = np.asarray(train_target_vel)[best]
    return np.ascontiguousarray(out)


def kernel(in_vel, train_obs_vel, train_target_vel):
    nc = _get_nc()
    in_maps = make_in_maps(in_vel, train_obs_vel)
    res = run_bass_kernel_spmd(nc, in_maps, list(range(CORES)))
    return finish(res.results, in_vel, train_obs_vel, train_target_vel)


# revision 23
# speedup vs baseline: 1.2744x; 1.0389x over previous
"""Nearest-neighbor retrieval kernel for Trainium2 (8 NeuronCores, SPMD).

Problem: dis[i] = mean((in_vel - train_obs_vel[i])**2); return
train_target_vel[argmin(dis)].

Strategy: only train_obs_vel has to stream through the devices. The device
pass is a bf16 *screen* over a host-transposed layout XT[feature, sample]:
with features on partitions, the query q becomes a per-partition scalar, so

  - ScalarE  ACTIVATE(Square, bias=-q)           -> (x-q)^2 in ONE op
  - VectorE  TENSOR_SCALAR(sub) + TENSOR_TENSOR(mult)  (2x bf16 mode)
  - TensorE  ones-vector matmul, PSUM-accumulated over the 9 feature
             chunks                              -> sum over features

The feature reduction runs on the otherwise-idle TensorE instead of
costing VectorE/ScalarE passes, and the subtract folds into the ACT op.
The screen keys (~0.4% noise vs a ~0.2% min-gap margin) feed a host-side
exact f32 recheck of the top-1024 candidates, so the result is bit-exact.

Sharding: 12500 samples per core, padded to 12544; features padded
1056 -> chunks of 128x8 + 32. All big DMAs use 128-partition descriptor
lists (the HWDGE only engages all 16 SDMA engines at 128 partitions).
"""

import sys

sys.path.insert(0, "/opt/trn_rl_repo")

import ml_dtypes
import numpy as np

import concourse.bacc as bacc
import concourse.mybir as mybir
import concourse.tile as tile
from concourse.bass_utils import run_bass_kernel_spmd

# Problem shapes (hardcoded per harness contract)
N = 100000
T_OBS = 16
T_OUT = 25
D = 66
F = T_OBS * D  # 1056 features per sample
CORES = 8
PER = N // CORES  # 12500 samples per core
NS = 12544  # padded samples per core (128*98)
PAD_VAL = 1.0e4  # pad samples get a huge distance; never the argmin
NCH = 9  # feature chunks: 8 x 128 + 1 x 32
PCH = [128] * 8 + [32]
W = 448  # samples per PSUM block (one 2KB f32 bank)
NBLK = NS // W  # 28 blocks
BPG = 7  # blocks per DMA sample-group
DW = W * BPG  # 3136 samples per DMA group
NGRP = NBLK // BPG  # 4 groups
S_OF_9 = 5  # chunks per block whose square runs on ScalarE (rest VectorE)
TOPK = 1024  # host-side exact recheck pool

_f32 = mybir.dt.float32
_bf16 = mybir.dt.bfloat16
_bf16_np = ml_dtypes.bfloat16


def build_nc(s_of_9=S_OF_9):
    nc = bacc.Bacc("TRN2", target_bir_lowering=False, debug=False)
    xt = nc.dram_tensor("xt", [F, NS], _bf16, kind="ExternalInput")
    qn = nc.dram_tensor("qn", [128, NCH], _f32, kind="ExternalInput")  # -q
    qp = nc.dram_tensor("qp", [128, NCH], _f32, kind="ExternalInput")  # +q
    key_out = nc.dram_tensor("key", [1, NS], _f32, kind="ExternalOutput")

    with tile.TileContext(nc) as tc:
        with (
            tc.tile_pool(name="xin", bufs=2) as xpool,
            tc.tile_pool(name="qpool", bufs=1) as qpool,
            tc.tile_pool(name="sqpool", bufs=14) as sqpool,
            tc.tile_pool(name="dpool", bufs=4) as dpool,
            tc.tile_pool(name="kout", bufs=4) as kpool,
            tc.tile_pool(name="psum", bufs=4, space="PSUM") as ppool,
        ):
            qn_t = qpool.tile([128, NCH], _f32, tag="qn")
            nc.sync.dma_start(out=qn_t[:], in_=qn[:])
            qp_t = qpool.tile([128, NCH], _f32, tag="qp")
            nc.sync.dma_start(out=qp_t[:], in_=qp[:])
            ones = qpool.tile([128, 1], _bf16, tag="ones")
            nc.vector.memset(ones[:], 1.0)

            pair_i = 0
            for g in range(NGRP):
                # load all 9 feature-chunk strips for this sample group
                xts = []
                for c in range(NCH):
                    p = PCH[c]
                    xtile = xpool.tile([128, DW], _bf16, tag=f"x{c}")
                    nc.sync.dma_start(
                        out=xtile[0:p, :],
                        in_=xt[c * 128 : c * 128 + p, g * DW : (g + 1) * DW],
                    )
                    xts.append(xtile)

                # sq production in 2-block pairs (bigger ops); matmuls per block
                for b0, b1 in ((0, 2), (2, 4), (4, 6), (6, 7)):
                    lo, hi = b0 * W, b1 * W
                    ww = hi - lo
                    n_s = s_of_9
                    pair_i += 1
                    sqs = []
                    for c in range(NCH):
                        p = PCH[c]
                        xs = xts[c][0:p, lo:hi]
                        sq = sqpool.tile([128, 2 * W], _bf16, tag="sq")
                        if c < n_s:
                            # ScalarE: (x - q)^2 in one ACTIVATE
                            nc.scalar.activation(
                                out=sq[0:p, 0:ww],
                                in_=xs,
                                func=mybir.ActivationFunctionType.Square,
                                bias=qn_t[0:p, c : c + 1],
                            )
                        else:
                            # VectorE: subtract (2x) + square (2x)
                            d = dpool.tile([128, 2 * W], _bf16, tag="d")
                            nc.vector.tensor_scalar_sub(
                                d[0:p, 0:ww], xs, qp_t[0:p, c : c + 1]
                            )
                            nc.vector.tensor_mul(
                                sq[0:p, 0:ww], d[0:p, 0:ww], d[0:p, 0:ww]
                            )
                        sqs.append(sq)
                    for b in range(b0, b1):
                        slo = (b - b0) * W
                        # two parallel PSUM chains: S-chunks -> A, V-chunks -> B
                        psa = ppool.tile([1, W], _f32, tag="psa")
                        psb = ppool.tile([1, W], _f32, tag="psb")
                        for c in range(NCH):
                            p = PCH[c]
                            ps, group = (
                                (psa, (0, n_s - 1))
                                if c < n_s
                                else (psb, (n_s, NCH - 1))
                            )
                            nc.tensor.matmul(
                                ps[:],
                                ones[0:p, :],
                                sqs[c][0:p, slo : slo + W],
                                start=(c == group[0]),
                                stop=(c == group[1]),
                            )
                        # merge chains: ScalarE copies B out of PSUM (TT can
                        # only read one PSUM input), VectorE adds
                        ktb = kpool.tile([1, W], _f32, tag="ktb")
                        nc.scalar.copy(ktb[:], psb[:])
                        kt = kpool.tile([1, W], _f32, tag="kt")
                        nc.vector.tensor_add(kt[:], psa[:], ktb[:])
                        nc.gpsimd.dma_start(
                            out=key_out[:, g * DW + b * W : g * DW + (b + 1) * W],
                            in_=kt[:],
                        )
    nc.compile()
    return nc


_nc_cache = {}


def _get_nc():
    key = (S_OF_9,)
    if key not in _nc_cache:
        _nc_cache[key] = build_nc()
    return _nc_cache[key]


def make_in_maps(in_vel, train_obs_vel):
    q = np.asarray(in_vel, dtype=np.float32).reshape(F)
    qpad = np.zeros(128 * NCH, dtype=np.float32)
    qpad[:F] = q
    qcols = qpad.reshape(NCH, 128).T  # [128, NCH]
    qp_h = np.ascontiguousarray(qcols)
    qn_h = np.ascontiguousarray(-qcols)
    X = np.asarray(train_obs_vel, dtype=np.float32).reshape(N, F)
    in_maps = []
    for c in range(CORES):
        xb = np.full((F, NS), PAD_VAL, dtype=_bf16_np)
        xb[:, :PER] = X[c * PER : (c + 1) * PER].astype(_bf16_np).T
        in_maps.append({"xt": np.ascontiguousarray(xb), "qn": qn_h, "qp": qp_h})
    return in_maps


def finish(results, in_vel, train_obs_vel, train_target_vel):
    # keys[core][0, j] screens padded-local sample j (original column order)
    keys = np.stack([np.asarray(r["key"]) for r in results])  # [8, 1, NS]
    flat = keys.reshape(CORES, NS)[:, :PER].reshape(-1)  # drop pads
    k = min(TOPK, flat.size)
    cand = np.sort(np.argpartition(flat, k - 1)[:k])
    # exact f32 recheck of the screened candidates
    q = np.asarray(in_vel, dtype=np.float32).reshape(F)
    X = np.asarray(train_obs_vel, dtype=np.float32).reshape(N, F)
    d = X[cand] - q
    exact = np.einsum("ij,ij->i", d, d)
    best = int(cand[int(exact.argmin())])
    out = np.asarray(train_target_vel)[best]
    return np.ascontiguousarray(out)


def kernel(in_vel, train_obs_vel, train_target_vel):
    nc = _get_nc()
    in_maps = make_in_maps(in_vel, train_obs_vel)
    res = run_bass_kernel_spmd(nc, in_maps, list(range(CORES)))
    return finish(res.results, in_vel, train_obs_vel, train_target_vel)


# revision 25
# speedup vs baseline: 1.3302x; 1.0437x over previous
"""Nearest-neighbor retrieval kernel for Trainium2 (8 NeuronCores, SPMD).

Problem: dis[i] = mean((in_vel - train_obs_vel[i])**2); return
train_target_vel[argmin(dis)].

Strategy: only train_obs_vel has to stream through the devices. The device
pass is a bf16 *screen*: it computes approximate keys
k_i ~= sum((x_i - q)^2) for every sample (bf16 halves HBM traffic to
~26.5 MB/core and doubles VectorE throughput). The host then recomputes
exact f32 keys for the top-1024 screened candidates (~1 M flops) and picks
the true argmin — bf16 key noise is ~+-0.3 on a min-gap of ~4, so the true
argmin is inside the top-1024 with overwhelming margin, and the final
result is bit-exact.

Sharding: 12500 rows per core, padded to 12544 = 128*98 so DMA tiles have
128 partitions (the HWDGE only engages all 16 SDMA engines for
128-partition descriptor lists; at 125 partitions it uses 5 and runs 3x
slower). Per column (sample-slice) the engines split work:
  - VectorE  TENSOR_TENSOR(subtract) bf16 (2x mode)   -> diff
  - ScalarE  ACTIVATE(Square, accum_out)              -> key (most cols)
  - VectorE  TENSOR_TENSOR(mult) + TENSOR_REDUCE(add) -> key (offload cols)
so ScalarE (dtype-independent 1 elem/cycle) stops being the bottleneck.
The tiny [128, 98] f32 key tile per core returns to the host; no device
collectives are needed.
"""

import sys

sys.path.insert(0, "/opt/trn_rl_repo")

import ml_dtypes
import numpy as np

import concourse.bacc as bacc
import concourse.mybir as mybir
import concourse.tile as tile
from concourse.bass_utils import run_bass_kernel_spmd

# Problem shapes (hardcoded per harness contract)
N = 100000
T_OBS = 16
T_OUT = 25
D = 66
F = T_OBS * D  # 1056 features per sample
CORES = 8
PER = N // CORES  # 12500 samples per core
P = 128  # SBUF partitions
C = 98  # samples (columns) per partition
PER_PAD = P * C  # 12544 padded samples per core
PAD_VAL = 1.0e4  # pad rows get a huge distance; never the argmin
S = 2  # samples per partition per DMA tile
N_VCOLS = 19  # columns whose square+reduce runs on VectorE (mul+reduce)
N_GSUBS = 0  # GpSimd subtract was a net loss (port contention)
TOPK = 1024  # host-side exact recheck pool


def _spread(n, total=C):
    """n column indices spread evenly over [0, total)."""
    return {int(round(i * total / n)) % total for i in range(n)}

_f32 = mybir.dt.float32
_bf16 = mybir.dt.bfloat16
_bf16_np = ml_dtypes.bfloat16


def build_nc(s=S, xin_bufs=8, n_vcols=N_VCOLS, n_gsubs=N_GSUBS):
    ntiles = C // s
    assert ntiles * s == C
    vcols = _spread(n_vcols)
    gsubs = _spread(n_gsubs)
    nc = bacc.Bacc("TRN2", target_bir_lowering=False, debug=False)
    x = nc.dram_tensor("x", [PER_PAD, F], _bf16, kind="ExternalInput")
    qb = nc.dram_tensor("qb", [P, F], _bf16, kind="ExternalInput")
    key_out = nc.dram_tensor("key", [P, C], _f32, kind="ExternalOutput")

    # [12544, 1056] -> [128 partitions, 98*1056 contiguous bf16]
    xr = x[:].rearrange("(p c) d -> p (c d)", p=P)

    with tile.TileContext(nc) as tc:
        with (
            tc.tile_pool(name="xin", bufs=xin_bufs) as xpool,
            tc.tile_pool(name="qpool", bufs=1) as qpool,
            tc.tile_pool(name="scratch", bufs=4) as spool,
            tc.tile_pool(name="acc", bufs=1) as apool,
        ):
            q_tile = qpool.tile([P, F], _bf16)
            nc.sync.dma_start(out=q_tile[:], in_=qb[:])

            key_t = apool.tile([P, C], _f32)

            for t in range(ntiles):
                xt = xpool.tile([P, s * F], _bf16, tag="xt")
                dma_eng = nc.sync if t % 2 == 0 else nc.gpsimd
                dma_eng.dma_start(
                    out=xt[:], in_=xr[:, t * s * F : (t + 1) * s * F]
                )
                for j in range(s):
                    col = t * s + j
                    xs = xt[:, j * F : (j + 1) * F]
                    diff = spool.tile([P, F], _bf16, tag="diff")
                    sub_eng = nc.gpsimd if col in gsubs else nc.vector
                    sub_eng.tensor_sub(diff[:], xs, q_tile[:])
                    kcol = key_t[:, col : col + 1]
                    if col in vcols:
                        # VectorE path: mult + reduce
                        sq = spool.tile([P, F], _bf16, tag="vsq")
                        nc.vector.tensor_mul(sq[:], diff[:], diff[:])
                        nc.vector.tensor_reduce(
                            kcol,
                            sq[:],
                            axis=mybir.AxisListType.X,
                            op=mybir.AluOpType.add,
                        )
                    else:
                        # ScalarE path: Square with free-axis accumulate
                        sq = spool.tile([P, F], _bf16, tag="ssq")
                        nc.scalar.activation(
                            out=sq[:],
                            in_=diff[:],
                            func=mybir.ActivationFunctionType.Square,
                            accum_out=kcol,
                        )

            nc.sync.dma_start(out=key_out[:], in_=key_t[:])
    nc.compile()
    return nc


_nc_cache = {}


def _get_nc():
    key = (S, N_VCOLS, N_GSUBS)
    if key not in _nc_cache:
        _nc_cache[key] = build_nc()
    return _nc_cache[key]


def make_in_maps(in_vel, train_obs_vel):
    q = np.asarray(in_vel, dtype=np.float32).reshape(F)
    qbn = np.ascontiguousarray(
        np.broadcast_to(q.astype(_bf16_np), (P, F))
    )
    X = np.asarray(train_obs_vel, dtype=np.float32).reshape(N, F)
    Xb = X.astype(_bf16_np)
    in_maps = []
    for c in range(CORES):
        xp = np.full((PER_PAD, F), PAD_VAL, dtype=_bf16_np)
        xp[:PER] = Xb[c * PER : (c + 1) * PER]
        in_maps.append({"x": xp, "qb": qbn})
    return in_maps


def finish(results, in_vel, train_obs_vel, train_target_vel):
    # keys[core][p, col] screens padded-local sample p*C + col; flattening
    # in C order reproduces the padded-local sample order.
    keys = np.stack([np.asarray(r["key"]) for r in results])  # [8, P, C]
    flat = keys.reshape(CORES, PER_PAD)[:, :PER].reshape(-1)  # drop pads
    k = min(TOPK, flat.size)
    cand = np.sort(np.argpartition(flat, k - 1)[:k])
    # exact f32 recheck of the screened candidates
    q = np.asarray(in_vel, dtype=np.float32).reshape(F)
    X = np.asarray(train_obs_vel, dtype=np.float32).reshape(N, F)
    d = X[cand] - q
    exact = np.einsum("ij,ij->i", d, d)
    best = int(cand[int(exact.argmin())])
    out = np.asarray(train_target_vel)[best]
    return np.ascontiguousarray(out)


def kernel(in_vel, train_obs_vel, train_target_vel):
    nc = _get_nc()
    in_maps = make_in_maps(in_vel, train_obs_vel)
    res = run_bass_kernel_spmd(nc, in_maps, list(range(CORES)))
    return finish(res.results, in_vel, train_obs_vel, train_target_vel)
